# revision 60
# baseline (speedup 1.0000x reference)
"""LocalAutoCorr2D Trainium2 kernel.

out[b,c,i,j,dy,dx] = sum_{y,x valid} x[b,c,4i+y,4j+x] * x[b,c,4i+y+sy,4j+x+sx]
with (sy,sx) = (dy-4, dx-4), 8x8 windows at stride 4 on a 96x96 image,
zero-padded at window boundaries.  Batch-sharded over 8 NeuronCores
(one batch item per core); ~127 us HW exec.

Device layout (host pre-packs it): superimage rows g = 96*(c div 16) + h,
g in [0,384); partition p = g mod 128, free dim = (beta = g div 128,
r = w mod 4, ap = w div 4 + 1, c16 = c mod 16), zero pads at ap=0,25.
FD = 3*4*26*16 = 4992 fp16 elems per partition.

Pipeline per canonical shift (40 classes after out[s] == out[-s]):
  - product Q = X0 .* T_sy on the Vector engine as per-(r-phase) 3D-AP
    ops over data columns; the phase-major layout turns the horizontal
    shift sx into a flat free-dim offset (phase r reads phase r+sx with
    a one-block carry lam, so all operands stay contiguous for the DVE
    2x_1P mode).  The T_sy tiles (partition-shift by sy) are packed
    host-side along with X0 and DMA-loaded from DRAM just-in-time, one
    sy-group ahead (on-device SBUF->SBUF copies run at only ~25 GB/s).
  - vertical 8-row box-sum: matmuls with 0/1 window matrices contracting
    the partition dim; the horizontal box-sum folds into PSUM
    accumulation over the 8-|sx| in-window x offsets.  Four independent
    accumulation chains (one per channel-group c4) on distinct col-tile
    positions (tile_position=(0,32*c4)) overlap on the PE quadrants;
    chains for c4=1,2 take two matmuls per step (their rows straddle
    two beta blocks).  N=384 fully-contiguous rhs per matmul.
  - PSUM -> SBUF on Scalar (window matrices padded to M=32 so all 128
    PSUM rows are written), one DMA per shift of the 40 canonical
    cells; the 24 mirror cells are replicated host-side.
"""

import functools
import os
import sys

import numpy as np

sys.path.insert(0, "/opt/trn_rl_repo")

import concourse.bacc as bacc  # noqa: E402
import concourse.mybir as mybir  # noqa: E402
from concourse import bass_utils  # noqa: E402
from concourse.tile import TileContext  # noqa: E402

B, C, H, W = 8, 64, 96, 96
NH = NW = 23
NCORES = 8
P = 128
NB = 3            # beta blocks of 128 superrows
C4, C16 = 4, 16   # channel groups x channels-per-group
APD = 26          # padded a-dim: ap = a+1, zeros at ap=0,25
FR = APD * C16    # 416: elems per (phase r) block
FB = 4 * FR       # 1664: elems per beta block
FD = NB * FB      # 4992: total free dim

fp32 = mybir.dt.float32
fp16 = mybir.dt.float16

def _canonical_cells():
    """Map canonical shift (sy>=0, sx) -> list of output cells (dy,dx)."""
    cells = {}
    for dy in range(8):
        for dx in range(8):
            sy, sx = dy - 4, dx - 4
            key = (sy, sx) if (sy > 0 or (sy == 0 and sx >= 0)) else (-sy, -sx)
            cells.setdefault(key, []).append((dy, dx))
    assert len(cells) == 40
    return cells


CELLS = _canonical_cells()
ORDER = sorted(CELLS.keys(), key=lambda s: (s[0], abs(s[1])))


def _amat_np():
    """Vertical box-sum weights (192 cols per sy, 6 blocks of 32 =
    per-(c4,beta) window matrices), followed by partition-shift
    permutation matrices for building T_sy on the PE:
      col 960 + (sy-1)*256 + [0:128)   S_sy  [p, m] = (p == m + sy)
      col 960 + (sy-1)*256 + [128:256) S2_sy [p, m] = (p == m + sy - 128)
    """
    a = np.zeros((P, 5 * 192 + 4 * 256), np.float16)
    blocks = [(0, 0, 0), (0, 1, 32), (2, 2, 64), (2, 3, 96),
              (1, 1, 128), (1, 2, 160)]
    for sy in range(5):
        for beta, c4, cb in blocks:
            base = sy * 192 + cb
            for p in range(P):
                g = 128 * beta + p
                if g // 96 != c4:
                    continue
                h = g % 96
                for i in range(NH):
                    if 0 <= h - 4 * i < 8 - sy:
                        a[p, base + i] = 1.0
    for sy in range(1, 5):
        s0 = 960 + (sy - 1) * 256
        for m in range(P):
            if m + sy < P:
                a[m + sy, s0 + m] = 1.0
            else:
                a[m + sy - P, s0 + 128 + m] = 1.0
    return a


def _pack_np(xb):
    """Host-side de-interleave of one batch item into the packed
    phase-major layout plus its four row-shifted copies T_sy, each
    [P, FD] fp16 (pad cols ap=0,25 zeroed; shifted-out rows zero)."""
    v = xb.reshape(4, 16, 96, 24, 4)          # (c4, c16, h, a, r)
    g = np.zeros((384, 4, APD, C16), np.float16)
    g[:, :, 1:25, :] = (
        v.transpose(0, 2, 4, 3, 1).reshape(384, 4, 24, 16)
    )
    g = g.reshape(384, FD // NB)

    def fold(garr):
        # device free-dim order (r, beta, ap, c16): per-phase slices are
        # flat-contiguous across the three beta blocks
        return np.ascontiguousarray(
            garr.reshape(NB, P, 4, FR).transpose(1, 2, 0, 3).reshape(P, FD)
        )

    tiles = {"x0": fold(g)}
    for sy in range(1, 5):
        gs = np.zeros_like(g)
        gs[:384 - sy] = g[sy:]
        tiles[f"t{sy}"] = fold(gs)
    return tiles


def build_nc():
    nc = bacc.Bacc()
    x0_dram = nc.dram_tensor("x0", [P, FD], fp16, kind="ExternalInput")
    t_dram = {sy: nc.dram_tensor(f"t{sy}", [P, FD], fp16,
                                 kind="ExternalInput") for sy in range(1, 5)}
    amat_dram = nc.dram_tensor("amat", [P, 5 * 192 + 4 * 256], fp16,
                               kind="ExternalInput")
    out_dram = nc.dram_tensor("out", [40, P, NH * C16], fp32,
                              kind="ExternalOutput")

    with TileContext(nc) as tc:
        with (
            tc.tile_pool(name="const", bufs=1) as cpool,
            tc.tile_pool(name="xt", bufs=1) as tpool,
            tc.tile_pool(name="q", bufs=4) as qpool,
            tc.tile_pool(name="o", bufs=6) as opool,
            tc.tile_pool(name="ps", bufs=8, space="PSUM") as ppool,
        ):
            # ---- X0 arrives pre-packed from the host; issued first so
            # the first products start as early as possible, half a beta
            # block per DMA piece
            x0 = tpool.tile([P, FD], fp16)
            for piece in range(2 * NB):
                hf = FB // 2
                nc.sync.dma_start(
                    x0[:, piece * hf:(piece + 1) * hf],
                    x0_dram[:, piece * hf:(piece + 1) * hf],
                )
            amat_t = cpool.tile([P, 5 * 192 + 4 * 256], fp16)
            nc.scalar.dma_start(amat_t, amat_dram[:, :])

            # ---- row-shifted copies T_sy come pre-packed from the host
            tt = {0: x0}

            def build_t(sy, on_pe=False):
                t = tpool.tile([P, FD], fp16, name=f"T{sy}")
                tt[sy] = t
                for beta in range(NB):
                    nc.sync.dma_start(
                        t[:, beta * FB:(beta + 1) * FB],
                        t_dram[sy][:, beta * FB:(beta + 1) * FB],
                    )

            # ---- per-shift: product -> col-tiled matmuls -> copy -> DMA out
            sy_seen = set()
            for ks, (sy, sx) in enumerate(ORDER):
                eng = nc.vector
                q = qpool.tile([P, FD], fp16, tag="q")
                RB = NB * FR  # 1248: flat elems per phase r

                # flat per-phase products: out phase r multiplies in1
                # phase r' = r + sx - 4*lam at inner offset 16*lam; block
                # pads supply the zero columns the lam-carry reads, and
                # the two OOB edge cols (never consumed, or consumed only
                # by the discarded j=23 output) are trimmed.
                for r in range(4):
                    if sx >= 0:
                        lam = 0 if r < 4 - sx else 1
                    else:
                        lam = -1 if r < -sx else 0
                    rp = r + sx - 4 * lam
                    o0, i0, ln = r * RB, rp * RB + lam * C16, RB
                    if lam == 1 and rp == 3:
                        ln -= C16
                    elif lam == -1 and rp == 0:
                        o0, i0, ln = o0 + C16, 0, ln - C16
                    eng.tensor_mul(
                        q[:, o0:o0 + ln],
                        x0[:, o0:o0 + ln],
                        tt[sy][:, i0:i0 + ln],
                    )

                # matmuls: 4 independent accumulation chains, one per c4
                # (PSUM rows [32*c4, +32), col-tile position 32*c4).
                # Chains never share PSUM rows, so the PE overlaps them;
                # emission order keeps adjacent mms on distinct positions.
                # (c4, beta, amat col): chain c4=1 has beta 0+1, c4=2 has 1+2
                mm_seq = [(0, 0, 0), (1, 0, 32), (2, 1, 160),
                          (3, 2, 96), (1, 1, 128), (2, 2, 64)]
                chain_first = {0: 0, 1: 1, 2: 2, 3: 3}
                chain_last = {0: 0, 1: 4, 2: 5, 3: 3}
                xlist = list(range(max(0, -sx), 8 - max(0, sx)))
                pt = ppool.tile([P, 384], fp32, tag="ps")
                for xi, xx in enumerate(xlist):
                    rx, jjx = xx & 3, xx >> 2
                    for mi, (c4, bb, acol) in enumerate(mm_seq):
                        rhs_off = rx * RB + bb * FR + (jjx + 1) * C16
                        nc.tensor.matmul(
                            pt[32 * c4:32 * c4 + 32, :],
                            amat_t[:, sy * 192 + acol:sy * 192 + acol + 32],
                            q[:, rhs_off:rhs_off + 384],
                            start=(xi == 0 and chain_first[c4] == mi),
                            stop=(xi == len(xlist) - 1
                                  and chain_last[c4] == mi),
                            tile_position=(0, 32 * c4),
                            skip_group_check=True,
                        )

                o_t = opool.tile([P, 384], fp32, tag="o")
                nc.scalar.copy(o_t, pt)
                nc.sync.dma_start(out_dram[ks], o_t[:, 0:NH * C16])
                if sy not in sy_seen:
                    sy_seen.add(sy)
                    if sy + 1 <= 4:
                        build_t(sy + 1, on_pe=True)

    if not nc.is_finalized():
        nc.finalize()
    return nc


@functools.lru_cache(maxsize=1)
def _get_nc():
    return build_nc()


def _run(x, trace=False):
    amat = _amat_np()
    nc = _get_nc()
    in_maps = [dict(_pack_np(x[b]), amat=amat) for b in range(NCORES)]
    return bass_utils.run_bass_kernel_spmd(
        nc, in_maps, core_ids=list(range(NCORES)), trace=trace,
    )


def kernel(**inputs) -> np.ndarray:
    x = np.asarray(inputs["x"], dtype=np.float32)
    assert x.shape == (B, C, H, W)
    res = _run(x, trace=bool(int(os.environ.get("KERNEL_TRACE", "0"))))
    outs = np.stack([r["out"] for r in res.results])  # [B, 40, 128, (j c16)]
    blk = outs.reshape(B, 40, C4, 32, NH, C16)[:, :, :, :NH]
    blk = blk.transpose(0, 1, 2, 5, 3, 4).reshape(B, 40, C, NH, NH)
    full = np.empty((B, C, NH, NH, 8, 8), np.float32)
    for ks, key in enumerate(ORDER):
        for dy, dx in CELLS[key]:
            full[:, :, :, :, dy, dx] = blk[:, ks]
    return full


if __name__ == "__main__":
    rng = np.random.default_rng(0)
    x = rng.standard_normal((B, C, H, W), dtype=np.float32)
    y = kernel(x=x)
    print("out", y.shape, y.dtype, float(np.abs(y).max()))


# revision 61
# speedup vs baseline: 1.0576x; 1.0576x over previous
"""LocalAutoCorr2D Trainium2 kernel.

out[b,c,i,j,dy,dx] = sum_{y,x valid} x[b,c,4i+y,4j+x] * x[b,c,4i+y+sy,4j+x+sx]
with (sy,sx) = (dy-4, dx-4), 8x8 windows at stride 4 on a 96x96 image,
zero-padded at window boundaries.  Batch-sharded over 8 NeuronCores
(one batch item per core); ~127 us HW exec.

Device layout (host pre-packs it): superimage rows g = 96*(c div 16) + h,
g in [0,384); partition p = g mod 128, free dim = (beta = g div 128,
r = w mod 4, ap = w div 4 + 1, c16 = c mod 16), zero pads at ap=0,25.
FD = 3*4*26*16 = 4992 fp16 elems per partition.

Pipeline per canonical shift (40 classes after out[s] == out[-s]):
  - product Q = X0 .* T_sy on the Vector engine as per-(r-phase) 3D-AP
    ops over data columns; the phase-major layout turns the horizontal
    shift sx into a flat free-dim offset (phase r reads phase r+sx with
    a one-block carry lam, so all operands stay contiguous for the DVE
    2x_1P mode).  The T_sy tiles (partition-shift by sy) are packed
    host-side along with X0 and DMA-loaded from DRAM just-in-time, one
    sy-group ahead (on-device SBUF->SBUF copies run at only ~25 GB/s).
  - vertical 8-row box-sum: matmuls with 0/1 window matrices contracting
    the partition dim; the horizontal box-sum folds into PSUM
    accumulation over the 8-|sx| in-window x offsets.  Four independent
    accumulation chains (one per channel-group c4) on distinct col-tile
    positions (tile_position=(0,32*c4)) overlap on the PE quadrants;
    chains for c4=1,2 take two matmuls per step (their rows straddle
    two beta blocks).  N=384 fully-contiguous rhs per matmul.
  - PSUM -> SBUF on Scalar (window matrices padded to M=32 so all 128
    PSUM rows are written), one DMA per shift of the 40 canonical
    cells; the 24 mirror cells are replicated host-side.
"""

import functools
import os
import sys

import numpy as np

sys.path.insert(0, "/opt/trn_rl_repo")

import concourse.bacc as bacc  # noqa: E402
import concourse.mybir as mybir  # noqa: E402
from concourse import bass_utils  # noqa: E402
from concourse.tile import TileContext  # noqa: E402

B, C, H, W = 8, 64, 96, 96
NH = NW = 23
NCORES = 8
P = 128
NB = 3            # beta blocks of 128 superrows
C4, C16 = 4, 16   # channel groups x channels-per-group
APD = 26          # padded a-dim: ap = a+1, zeros at ap=0,25
FR = APD * C16    # 416: elems per (phase r) block
FB = 4 * FR       # 1664: elems per beta block
FD = NB * FB      # 4992: total free dim

fp32 = mybir.dt.float32
fp16 = mybir.dt.float16

def _canonical_cells():
    """Map canonical shift (sy>=0, sx) -> list of output cells (dy,dx)."""
    cells = {}
    for dy in range(8):
        for dx in range(8):
            sy, sx = dy - 4, dx - 4
            key = (sy, sx) if (sy > 0 or (sy == 0 and sx >= 0)) else (-sy, -sx)
            cells.setdefault(key, []).append((dy, dx))
    assert len(cells) == 40
    return cells


CELLS = _canonical_cells()
ORDER = sorted(CELLS.keys(), key=lambda s: (s[0], abs(s[1])))


def _amat_np():
    """Vertical box-sum weights (192 cols per sy, 6 blocks of 32 =
    per-(c4,beta) window matrices), followed by partition-shift
    permutation matrices for building T_sy on the PE:
      col 960 + (sy-1)*256 + [0:128)   S_sy  [p, m] = (p == m + sy)
      col 960 + (sy-1)*256 + [128:256) S2_sy [p, m] = (p == m + sy - 128)
    """
    a = np.zeros((P, 5 * 192 + 4 * 256), np.float16)
    blocks = [(0, 0, 0), (0, 1, 32), (2, 2, 64), (2, 3, 96),
              (1, 1, 128), (1, 2, 160)]
    for sy in range(5):
        for beta, c4, cb in blocks:
            base = sy * 192 + cb
            for p in range(P):
                g = 128 * beta + p
                if g // 96 != c4:
                    continue
                h = g % 96
                for i in range(NH):
                    if 0 <= h - 4 * i < 8 - sy:
                        a[p, base + i] = 1.0
    for sy in range(1, 5):
        s0 = 960 + (sy - 1) * 256
        for m in range(P):
            if m + sy < P:
                a[m + sy, s0 + m] = 1.0
            else:
                a[m + sy - P, s0 + 128 + m] = 1.0
    return a


def _pack_np(xb):
    """Host-side de-interleave of one batch item into the packed
    phase-major layout plus its four row-shifted copies T_sy, each
    [P, FD] fp16 (pad cols ap=0,25 zeroed; shifted-out rows zero)."""
    v = xb.reshape(4, 16, 96, 24, 4)          # (c4, c16, h, a, r)
    g = np.zeros((384, 4, APD, C16), np.float16)
    g[:, :, 1:25, :] = (
        v.transpose(0, 2, 4, 3, 1).reshape(384, 4, 24, 16)
    )
    g = g.reshape(384, FD // NB)

    def fold(garr):
        return np.ascontiguousarray(
            garr.reshape(NB, P, FD // NB).transpose(1, 0, 2).reshape(P, FD)
        )

    tiles = {"x0": fold(g)}
    for sy in range(1, 5):
        gs = np.zeros_like(g)
        gs[:384 - sy] = g[sy:]
        tiles[f"t{sy}"] = fold(gs)
    return tiles


def build_nc():
    nc = bacc.Bacc()
    x0_dram = nc.dram_tensor("x0", [P, FD], fp16, kind="ExternalInput")
    t_dram = {sy: nc.dram_tensor(f"t{sy}", [P, FD], fp16,
                                 kind="ExternalInput") for sy in range(1, 5)}
    amat_dram = nc.dram_tensor("amat", [P, 5 * 192 + 4 * 256], fp16,
                               kind="ExternalInput")
    out_dram = nc.dram_tensor("out", [40, P, NH * C16], fp32,
                              kind="ExternalOutput")

    with TileContext(nc) as tc:
        with (
            tc.tile_pool(name="const", bufs=1) as cpool,
            tc.tile_pool(name="xt", bufs=1) as tpool,
            tc.tile_pool(name="q", bufs=4) as qpool,
            tc.tile_pool(name="o", bufs=6) as opool,
            tc.tile_pool(name="ps", bufs=8, space="PSUM") as ppool,
        ):
            # ---- X0 arrives pre-packed from the host; issued first so
            # the first products start as early as possible, half a beta
            # block per DMA piece
            x0 = tpool.tile([P, FD], fp16)
            for piece in range(2 * NB):
                hf = FB // 2
                nc.sync.dma_start(
                    x0[:, piece * hf:(piece + 1) * hf],
                    x0_dram[:, piece * hf:(piece + 1) * hf],
                )
            amat_t = cpool.tile([P, 5 * 192 + 4 * 256], fp16)
            nc.scalar.dma_start(amat_t, amat_dram[:, :])

            # ---- row-shifted copies T_sy come pre-packed from the host
            tt = {0: x0}

            def build_t(sy, on_pe=False):
                t = tpool.tile([P, FD], fp16, name=f"T{sy}")
                tt[sy] = t
                for beta in range(NB):
                    nc.sync.dma_start(
                        t[:, beta * FB:(beta + 1) * FB],
                        t_dram[sy][:, beta * FB:(beta + 1) * FB],
                    )

            # ---- per-shift: product -> col-tiled matmuls -> copy -> DMA out
            sy_seen = set()
            for ks, (sy, sx) in enumerate(ORDER):
                eng = nc.vector
                q = qpool.tile([P, FD], fp16, tag="q")
                q3 = q.rearrange("p (b f) -> p b f", b=NB)
                x03 = x0.rearrange("p (b f) -> p b f", b=NB)
                t3 = tt[sy].rearrange("p (b f) -> p b f", b=NB)

                q4 = q.rearrange("p (b r f) -> p b r f", b=NB, r=4)
                x04 = x0.rearrange("p (b r f) -> p b r f", b=NB, r=4)
                t4 = tt[sy].rearrange("p (b r f) -> p b r f", b=NB, r=4)

                # 4D-AP products over data columns only (ap in [1,25));
                # the pad columns of Q are never written: the only padded
                # rhs read (ap=25 at jjx=1) lands in output col j=23,
                # which is sliced off before the out-DMA. lam is the
                # horizontal block-carry: in1 phase r' = r + sx - 4*lam,
                # inner offset 16*lam.
                def prod(r, lam, bs=None):
                    b0, b1 = (bs, bs + 1) if bs is not None else (0, NB)
                    eng.tensor_mul(
                        q4[:, b0:b1, r, C16:25 * C16],
                        x04[:, b0:b1, r, C16:25 * C16],
                        t4[:, b0:b1, r + sx - 4 * lam,
                           (1 + lam) * C16:(25 + lam) * C16],
                    )

                def regions():
                    if sx >= 0:
                        if sx < 4:
                            yield 0, 4 - sx, 0
                        if sx > 0:
                            yield 4 - sx, 4, 1
                    else:
                        s = -sx
                        yield 0, s, -1
                        yield s, 4, 0

                # per-phase ops: finer deps let each phase's matmuls start
                # as soon as that phase's product lands
                for r0, r1, lam in regions():
                    for r in range(r0, r1):
                        if ks < 2:
                            for b in range(NB):
                                prod(r, lam, bs=b)
                        else:
                            prod(r, lam)

                # matmuls: 4 independent accumulation chains, one per c4
                # (PSUM rows [32*c4, +32), col-tile position 32*c4).
                # Chains never share PSUM rows, so the PE overlaps them;
                # emission order keeps adjacent mms on distinct positions.
                # (c4, beta, amat col): chain c4=1 has beta 0+1, c4=2 has 1+2
                mm_seq = [(0, 0, 0), (1, 0, 32), (2, 1, 160),
                          (3, 2, 96), (1, 1, 128), (2, 2, 64)]
                chain_first = {0: 0, 1: 1, 2: 2, 3: 3}
                chain_last = {0: 0, 1: 4, 2: 5, 3: 3}
                xlist = list(range(max(0, -sx), 8 - max(0, sx)))
                pt = ppool.tile([P, 384], fp32, tag="ps")
                for xi, xx in enumerate(xlist):
                    rx, jjx = xx & 3, xx >> 2
                    rhs_off = rx * FR + (jjx + 1) * C16
                    for mi, (c4, bb, acol) in enumerate(mm_seq):
                        nc.tensor.matmul(
                            pt[32 * c4:32 * c4 + 32, :],
                            amat_t[:, sy * 192 + acol:sy * 192 + acol + 32],
                            q3[:, bb, rhs_off:rhs_off + 384],
                            start=(xi == 0 and chain_first[c4] == mi),
                            stop=(xi == len(xlist) - 1
                                  and chain_last[c4] == mi),
                            tile_position=(0, 32 * c4),
                            skip_group_check=True,
                        )

                o_t = opool.tile([P, 384], fp32, tag="o")
                nc.scalar.copy(o_t, pt)
                nc.sync.dma_start(out_dram[ks], o_t[:, 0:NH * C16])
                if sy not in sy_seen:
                    sy_seen.add(sy)
                    if sy + 1 <= 4:
                        build_t(sy + 1, on_pe=True)

    if not nc.is_finalized():
        nc.finalize()
    return nc


@functools.lru_cache(maxsize=1)
def _get_nc():
    return build_nc()


def _run(x, trace=False):
    amat = _amat_np()
    nc = _get_nc()
    in_maps = [dict(_pack_np(x[b]), amat=amat) for b in range(NCORES)]
    return bass_utils.run_bass_kernel_spmd(
        nc, in_maps, core_ids=list(range(NCORES)), trace=trace,
    )


def kernel(**inputs) -> np.ndarray:
    x = np.asarray(inputs["x"], dtype=np.float32)
    assert x.shape == (B, C, H, W)
    res = _run(x, trace=bool(int(os.environ.get("KERNEL_TRACE", "0"))))
    outs = np.stack([r["out"] for r in res.results])  # [B, 40, 128, (j c16)]
    blk = outs.reshape(B, 40, C4, 32, NH, C16)[:, :, :, :NH]
    blk = blk.transpose(0, 1, 2, 5, 3, 4).reshape(B, 40, C, NH, NH)
    full = np.empty((B, C, NH, NH, 8, 8), np.float32)
    for ks, key in enumerate(ORDER):
        for dy, dx in CELLS[key]:
            full[:, :, :, :, dy, dx] = blk[:, ks]
    return full


if __name__ == "__main__":
    rng = np.random.default_rng(0)
    x = rng.standard_normal((B, C, H, W), dtype=np.float32)
    y = kernel(x=x)
    print("out", y.shape, y.dtype, float(np.abs(y).max()))
